# revision 25
# baseline (speedup 1.0000x reference)
"""Trainium2 Bass kernel for nn_BasicBlock (binarized ResNet basic block).

Strategy (8 NeuronCores, batch-parallel, 1 image per core):
  - Both binarized 3x3 convs run as 9-tap shift matmuls (bf16, exact: all
    values are +-1/0 and accumulation is f32 in PSUM).
  - The T=64-chunked psum quantization q = 2*round_half_even(p/2) is the
    identity for even p; chunk parity is data-independent and odd only at
    the 124 border pixels of each 32x32 image.  Border pixels are fixed up
    with per-chunk matmuls whose rounding is done exactly in hardware by
    bracketing each chunk's PSUM accumulation with +K / -K matmuls
    (K = 1.5*2^24 forces f32 RNE rounding to multiples of 2).
  - Training-mode BN stats are all-reduced across the 8 cores (2KB payload).
  - conv2's binarized input is sign(v1 - mean1) (exact: v1 is integer and
    mean1 = sum/2^13 is exact in f32), computed on device with the ACT Sign
    LUT; its border im2col patches are built on device via PE transposes.
The overflow-rate output r1+r2 is provably always 0 (|round(p/2)| <= 32).
"""
import sys
sys.path.insert(0, '/opt/trn_rl_repo')
import numpy as np
import ml_dtypes

import concourse.bass as bass
import concourse.bacc as bacc
import concourse.mybir as mybir
import concourse.tile as tile
from concourse.bass_utils import run_bass_kernel_spmd
from concourse.masks import make_identity

FP = mybir.dt.float32
BF = mybir.dt.bfloat16
AF = mybir.ActivationFunctionType
ALU = mybir.AluOpType
MAGIC = float(np.float32(1.5 * 2 ** 24))
EPS_SIGN = float(2 ** -15)
N_CORES = 8
C = 256
HW = 1024
PADW = 34
NPIX = PADW * PADW          # 1156
WIN0 = 35                   # offset of out pixel (0,0) in padded coords
G = 36                      # psum chunks
# pixel row blocks for the main conv (psum bank <= 512 f32)
BLOCKS = [(0, 12), (12, 24), (24, 32)]
NBORD = 124

_cached = {}


def _window(blk):
    r0, r1 = blk
    off = r0 * PADW
    ln = (r1 - r0 - 1) * PADW + 32
    return off, ln


def _build_nc():
    nc = bacc.Bacc("TRN2", target_bir_lowering=False, debug=False,
                   num_devices=N_CORES)
    # ---- IO ----
    xpad_d = nc.dram_tensor("xpad", [2, 128, NPIX], BF, kind="ExternalInput")
    xres_d = nc.dram_tensor("xres", [2, 128, HW], FP, kind="ExternalInput")
    p1_d = nc.dram_tensor("p1", [128, 18 * 128], BF, kind="ExternalInput")
    w1m_d = nc.dram_tensor("w1m", [2, 128, 9 * 256], BF, kind="ExternalInput")
    w1c_d = nc.dram_tensor("w1c", [128, 18 * 256], BF, kind="ExternalInput")
    w2m_d = nc.dram_tensor("w2m", [2, 128, 9 * 256], BF, kind="ExternalInput")
    w2c_d = nc.dram_tensor("w2c", [128, 18 * 256], BF, kind="ExternalInput")
    bnp_d = nc.dram_tensor("bnp", [128, 8], FP, kind="ExternalInput")
    out_d = nc.dram_tensor("out", [2, 128, HW], FP, kind="ExternalOutput")

    with tile.TileContext(nc) as tc:
        with tc.tile_pool(name="big", bufs=1) as big, \
             tc.tile_pool(name="mainps", bufs=2, space="PSUM") as mainps, \
             tc.tile_pool(name="bordps", bufs=4, space="PSUM") as bordps, \
             tc.tile_pool(name="trps", bufs=1, space="PSUM") as trps, \
             tc.tile_pool(name="dram", bufs=1, space="DRAM") as drp, \
             tc.tile_pool(name="winp", bufs=2) as winp:

            # ---- constants ----
            ident = big.tile([128, 128], BF)
            make_identity(nc, ident[:])
            identf = big.tile([128, 128], FP)
            make_identity(nc, identf[:])
            ones1 = big.tile([1, 128], BF)
            krow = big.tile([1, 256], BF)
            nkrow = big.tile([1, 256], BF)
            epst = big.tile([128, 1], FP)
            nc.gpsimd.memset(ones1[:], 1.0)
            nc.gpsimd.memset(krow[:], MAGIC)
            nc.gpsimd.memset(nkrow[:], -MAGIC)
            nc.gpsimd.memset(epst[:], 1e-5)

            # ---- inputs to SBUF ----
            xpad = big.tile([128, 2 * NPIX], BF)
            xres = big.tile([128, 2 * HW], FP)
            p1 = big.tile([128, 18 * 128], BF)
            w1m = big.tile([128, 2 * 2304], BF)
            w1c = big.tile([128, 18 * 256], BF)
            w2m = big.tile([128, 2 * 2304], BF)
            w2c = big.tile([128, 18 * 256], BF)
            bnp = big.tile([128, 8], FP)
            for kt in range(2):
                nc.sync.dma_start(xpad[:, kt * NPIX:(kt + 1) * NPIX], xpad_d[kt])
                nc.sync.dma_start(w1m[:, kt * 2304:(kt + 1) * 2304], w1m_d[kt])
                nc.sync.dma_start(w2m[:, kt * 2304:(kt + 1) * 2304], w2m_d[kt])
                nc.sync.dma_start(xres[:, kt * HW:(kt + 1) * HW], xres_d[kt])
            nc.sync.dma_start(p1[:], p1_d[:])
            nc.sync.dma_start(w1c[:], w1c_d[:])
            nc.sync.dma_start(w2c[:], w2c_d[:])
            nc.sync.dma_start(bnp[:], bnp_d[:])

            v1 = big.tile([128, 2 * NPIX], BF)
            b2 = big.tile([128, 2 * NPIX], BF)
            v2 = big.tile([128, 2 * NPIX], BF)
            nc.gpsimd.memset(b2[:], 0.0)

            def conv(src, wm, wc, pat, vout):
                """src: [128, 2*NPIX] bf16 padded +-1/0; wm main wts; wc chunk wts;
                pat: chunk patches [128, 18*128]; vout: clipped int out (window)."""
                # main shift conv
                for ot in range(2):
                    for bi, blk in enumerate(BLOCKS):
                        off, ln = _window(blk)
                        ps = mainps.tile([128, 406], FP, tag="mainps")
                        n = 0
                        for kt in range(2):
                            for tap in range(9):
                                kh, kw = tap // 3, tap % 3
                                toff = kh * PADW + kw
                                nc.tensor.matmul(
                                    ps[:, 0:ln],
                                    wm[:, kt * 2304 + tap * 256 + ot * 128:
                                           kt * 2304 + tap * 256 + ot * 128 + 128],
                                    src[:, kt * NPIX + off + toff:
                                           kt * NPIX + off + toff + ln],
                                    start=(n == 0), stop=(n == 17))
                                n += 1
                        # evacuate with clip to [-128, 127]
                        nc.vector.tensor_scalar(
                            vout[:, ot * NPIX + WIN0 + off:ot * NPIX + WIN0 + off + ln],
                            ps[:, 0:ln], 127.0, -128.0, ALU.min, ALU.max)
                # Border: per-chunk psum in a FRESH psum slot, then one fused DVE
                # op per chunk does the exact quantization and accumulation:
                #   acc <- (psum +- K) + acc   (K sign alternates; (p+K) rounds the
                # chunk psum alone to 2*round_half_even(p/2)+K, and the K offsets
                # cancel pairwise, keeping acc exact).
                accs = [big.tile([128, 256], FP, tag=f"bacc{i}", name=f"bacc{i}")
                        for i in range(2)]
                nc.vector.memset(accs[1][:], 0.0)
                for g in range(G):
                    po = (g % 2) * 64
                    bp = bordps.tile([128, 256], FP, tag="bordps")
                    nc.tensor.matmul(bp[:], pat[po:po + 64, (g // 2) * 128:(g // 2) * 128 + 128],
                                     wc[po:po + 64, (g // 2) * 256:(g // 2) * 256 + 256],
                                     start=True, stop=True)
                    nc.vector.scalar_tensor_tensor(
                        accs[g % 2][:], bp[:], MAGIC if g % 2 == 0 else -MAGIC,
                        accs[1 - g % 2][:], ALU.add, ALU.add)
                baccs = accs[1]  # G even -> last write in accs[1], K offset cancelled
                # transpose [pix, cout] -> [cout, pix] and clip-write into vout borders
                for ct in range(2):
                    tp = trps.tile([128, 128], FP, tag="tr")
                    nc.tensor.transpose(tp[:], baccs[:, ct * 128:(ct + 1) * 128], identf[:])
                    base = ct * NPIX + WIN0
                    segs = [
                        (tp[:, 0:32], vout[:, base:base + 32]),                      # top
                        (tp[:, 32:64], vout[:, base + 31 * PADW:base + 31 * PADW + 32]),  # bottom
                        (tp[:, 64:94],
                         vout[:, base + PADW:base + PADW + 30 * PADW]
                         .rearrange("p (i j) -> p i j", j=PADW)[:, :, 0:1]
                         .rearrange("p i j -> p (i j)")),                            # left
                        (tp[:, 94:124],
                         vout[:, base + PADW:base + PADW + 30 * PADW]
                         .rearrange("p (i j) -> p i j", j=PADW)[:, :, 31:32]
                         .rearrange("p i j -> p (i j)")),                            # right
                    ]
                    for s, d in segs:
                        nc.vector.tensor_scalar(d, s, 127.0, -128.0, ALU.min, ALU.max)

            def stats_of(vt, statpack, use_z=None):
                """statpack [128,4]: sums & sumsq per kt. vt: [128, 2*NPIX] window
                tile (bf16) unless use_z given ([128, 2*HW] f32 compact)."""
                sq = big.tile([128, HW], FP, tag="sq")
                for kt in range(2):
                    if use_z is not None:
                        valid = use_z[:, kt * HW:(kt + 1) * HW]
                    else:
                        valid = (vt[:, kt * NPIX + WIN0:kt * NPIX + WIN0 + 32 * PADW]
                                 .rearrange("p (i j) -> p i j", j=PADW)[:, :, 0:32])
                    nc.vector.tensor_reduce(statpack[:, kt:kt + 1], valid,
                                            mybir.AxisListType.XYZW, ALU.add)
                    sqv = sq[:] if use_z is not None else \
                        sq[:].rearrange("p (i j) -> p i j", j=32)
                    nc.scalar.activation(sqv, valid, AF.Square)
                    nc.vector.tensor_reduce(statpack[:, 2 + kt:3 + kt], sq[:],
                                            mybir.AxisListType.XYZW, ALU.add)

            def allreduce(statpack, gstats, idx):
                cin = drp.tile([128, 4], FP, tag=f"cin{idx}")
                cout2 = drp.tile([N_CORES * 128, 4], FP, tag=f"cout{idx}",
                                 addr_space="Shared", name=f"cout{idx}")
                nc.sync.dma_start(cin[:], statpack[:])
                nc.gpsimd.collective_compute(
                    "AllGather", ALU.bypass, ins=[cin[:]], outs=[cout2[:]],
                    replica_groups=[list(range(N_CORES))])
                gath = big.tile([128, 8 * 4], FP, tag=f"gath{idx}",
                                name=f"gath{idx}")
                nc.sync.dma_start(
                    gath[:].rearrange("p (f r) -> p f r", r=N_CORES),
                    cout2[:].rearrange("(r p) f -> p f r", p=128))
                nc.vector.tensor_reduce(gstats[:],
                                        gath[:].rearrange("p (f r) -> p f r", r=N_CORES),
                                        mybir.AxisListType.X, ALU.add)

            def bn_scalars(gstats, scal):
                """gstats [128,4] (sum kt, sumsq kt) -> scal [128,4]: mean kt0,kt1,
                rstd kt0,kt1 (rsqrt(var+eps), Newton-refined)."""
                t = big.tile([128, 10], FP, tag="bnt")
                nc.vector.tensor_scalar(scal[:, 0:2], gstats[:, 0:2], 1.0 / 8192, None, ALU.mult)
                nc.vector.tensor_scalar(t[:, 0:2], gstats[:, 2:4], 1.0 / 8192, None, ALU.mult)
                nc.vector.tensor_tensor(t[:, 2:4], scal[:, 0:2], scal[:, 0:2], ALU.mult)
                nc.vector.tensor_tensor(t[:, 0:2], t[:, 0:2], t[:, 2:4], ALU.subtract)  # var
                nc.scalar.activation(t[:, 2:4], t[:, 0:2], AF.Sqrt, bias=epst[:], scale=1.0)
                nc.vector.reciprocal(t[:, 4:6], t[:, 2:4])  # y0
                nc.vector.tensor_scalar(t[:, 6:8], t[:, 0:2], 1e-5, 0.5, ALU.add, ALU.mult)
                cur = 4
                for dst in (8, 2):  # reuse cols; final y in t[:, 2:4]
                    nc.vector.tensor_tensor(t[:, dst:dst + 2], t[:, cur:cur + 2],
                                            t[:, cur:cur + 2], ALU.mult)
                    nc.vector.tensor_tensor(t[:, dst:dst + 2], t[:, 6:8],
                                            t[:, dst:dst + 2], ALU.mult)
                    nc.vector.tensor_scalar(t[:, dst:dst + 2], t[:, dst:dst + 2],
                                            -1.0, 1.5, ALU.mult, ALU.add)
                    nc.vector.tensor_tensor(t[:, dst:dst + 2] if dst != 2 else t[:, 2:4],
                                            t[:, cur:cur + 2], t[:, dst:dst + 2], ALU.mult)
                    cur = dst
                nc.vector.tensor_copy(scal[:, 2:4], t[:, 2:4])

            # =================== conv1 ===================
            conv(xpad, w1m, w1c, p1, v1)
            sp1 = big.tile([128, 4], FP, tag="sp1")
            stats_of(v1, sp1)
            gs1 = big.tile([128, 4], FP, tag="gs1")
            allreduce(sp1, gs1, 1)
            sc1 = big.tile([128, 4], FP, tag="sc1")
            bn_scalars(gs1, sc1)
            # b2 = Sign(scale1 * v1 + bias1); scale1 = gamma1*rstd, bias1 = beta1 - scale1*(m1 - eps)
            sc1v = big.tile([128, 4], FP, tag="sc1v")  # cols: scale kt0,kt1, bias kt0,kt1
            nc.vector.tensor_tensor(sc1v[:, 0:2], bnp[:, 0:2], sc1[:, 2:4], ALU.mult)
            tb = big.tile([128, 2], FP, tag="tb1")
            nc.vector.tensor_scalar(tb[:], sc1[:, 0:2], -1.0, EPS_SIGN, ALU.mult, ALU.add)
            nc.vector.tensor_tensor(tb[:], sc1v[:, 0:2], tb[:], ALU.mult)
            nc.vector.tensor_tensor(sc1v[:, 2:4], tb[:], bnp[:, 2:4], ALU.add)
            for kt in range(2):
                src = (v1[:, kt * NPIX + WIN0:kt * NPIX + WIN0 + 32 * PADW]
                       .rearrange("p (i j) -> p i j", j=PADW)[:, :, 0:32])
                dst = (b2[:, kt * NPIX + WIN0:kt * NPIX + WIN0 + 32 * PADW]
                       .rearrange("p (i j) -> p i j", j=PADW)[:, :, 0:32])
                nc.scalar.activation(dst, src, AF.Sign,
                                     bias=sc1v[:, 2 + kt:3 + kt], scale=sc1v[:, kt:kt + 1])

            # ============ conv2 border patches (device im2col) ============
            # For each tap, PE-transpose shifted pixel windows of b2 into
            # [border-pixel, channel] layout, packing 4 taps per PSUM tile.
            pT = big.tile([128, 2304], BF)  # patchesT2 [m, r], r = 9c+k
            nc.gpsimd.memset(pT[:], 0.0)
            pT_r = pT[:].rearrange("p (c k) -> p k c", k=9)
            for ct in range(2):
                a3 = (b2[:, ct * NPIX:(ct + 1) * NPIX]
                      .rearrange("p (r c) -> p r c", c=PADW))
                for k in range(9):
                    kh, kw = k // 3, k % 3
                    tmp = winp.tile([128, 128], BF, tag="wintmp")
                    nc.gpsimd.tensor_copy(
                        tmp[:, 0:64].rearrange("p (a b) -> p a b", b=32),
                        a3[:, kh:kh + 32:31, kw:kw + 32])
                    nc.vector.tensor_copy(
                        tmp[:, 64:124].rearrange("p (a b) -> p a b", b=30),
                        a3[:, 1 + kh:31 + kh, kw:kw + 32:31]
                        .rearrange("p r c -> p c r"))
                    tps = trps.tile([128, 128], BF, tag="tr")
                    nc.tensor.transpose(tps[:], tmp[:], ident[:])
                    nc.scalar.copy(pT_r[0:124, k, ct * 128:(ct + 1) * 128],
                                   tps[0:124, :])
            p2 = big.tile([128, 18 * 128], BF)
            for t in range(18):
                tp = trps.tile([128, 128], BF, tag="tr")
                nc.tensor.transpose(tp[:], pT[:, t * 128:(t + 1) * 128], ident[:])
                nc.vector.tensor_copy(p2[:, t * 128:(t + 1) * 128], tp[:])

            # =================== conv2 ===================
            conv(b2, w2m, w2c, p2, v2)
            # z = v2 + x (compact [128, 2*HW] f32)
            z = big.tile([128, 2 * HW], FP)
            for kt in range(2):
                valid = (v2[:, kt * NPIX + WIN0:kt * NPIX + WIN0 + 32 * PADW]
                         .rearrange("p (i j) -> p i j", j=PADW)[:, :, 0:32])
                nc.vector.tensor_tensor(
                    z[:, kt * HW:(kt + 1) * HW]
                    .rearrange("p (i j) -> p i j", j=32),
                    valid, xres[:, kt * HW:(kt + 1) * HW]
                    .rearrange("p (i j) -> p i j", j=32), ALU.add)
            sp2 = big.tile([128, 4], FP, tag="sp2")
            stats_of(None, sp2, use_z=z)
            gs2 = big.tile([128, 4], FP, tag="gs2")
            allreduce(sp2, gs2, 2)
            sc2 = big.tile([128, 4], FP, tag="sc2")
            bn_scalars(gs2, sc2)
            sc2v = big.tile([128, 4], FP, tag="sc2v")
            nc.vector.tensor_tensor(sc2v[:, 0:2], bnp[:, 4:6], sc2[:, 2:4], ALU.mult)
            nc.vector.tensor_tensor(sc2v[:, 2:4], sc2v[:, 0:2], sc2[:, 0:2], ALU.mult)
            nc.vector.tensor_tensor(sc2v[:, 2:4], bnp[:, 6:8], sc2v[:, 2:4], ALU.subtract)
            outt = big.tile([128, 2 * HW], FP)
            for kt in range(2):
                nc.vector.tensor_scalar(outt[:, kt * HW:(kt + 1) * HW],
                                        z[:, kt * HW:(kt + 1) * HW],
                                        sc2v[:, kt:kt + 1], sc2v[:, 2 + kt:3 + kt],
                                        ALU.mult, ALU.add)
                nc.vector.tensor_scalar(outt[:, kt * HW:(kt + 1) * HW],
                                        outt[:, kt * HW:(kt + 1) * HW],
                                        1.0, -1.0, ALU.min, ALU.max)
                nc.sync.dma_start(out_d[kt], outt[:, kt * HW:(kt + 1) * HW])

    nc.compile()
    return nc


# ---------------- host-side prep ----------------

def _border_indices():
    ii = np.concatenate([np.zeros(32, np.int64), np.full(32, 31, np.int64),
                         np.arange(1, 31), np.arange(1, 31)])
    jj = np.concatenate([np.arange(32), np.arange(32),
                         np.zeros(30, np.int64), np.full(30, 31, np.int64)])
    return ii, jj


def _prep_weights(w):
    ws = np.where(w >= 0, 1.0, -1.0).astype(np.float32)  # [o, c, kh, kw]
    wm = ws.transpose(1, 2, 3, 0).reshape(2, 128, 9 * 256)  # [c,(kh kw),o] -> kt split
    wflat = ws.reshape(256, 2304)          # [o, r]
    wcT = wflat.T                          # [r, o]
    wc = np.zeros((128, 18 * 256), np.float32)
    for g in range(G):
        wc[(g % 2) * 64:(g % 2) * 64 + 64, (g // 2) * 256:(g // 2) * 256 + 256] = \
            wcT[64 * g:64 * g + 64]
    return (wm.astype(ml_dtypes.bfloat16), wc.astype(ml_dtypes.bfloat16))


def _prep_core(xi, ii, jj):
    xb = np.where(xi >= 0, 1.0, -1.0).astype(np.float32)
    xpad = np.zeros((256, PADW, PADW), np.float32)
    xpad[:, 1:33, 1:33] = xb
    pat = np.empty((256, 9, NBORD), np.float32)
    for kh in range(3):
        for kw in range(3):
            pat[:, kh * 3 + kw, :] = xpad[:, ii + kh, jj + kw]
    patR = pat.reshape(2304, NBORD)
    p1 = np.zeros((128, 18 * 128), np.float32)
    for g in range(G):
        p1[(g % 2) * 64:(g % 2) * 64 + 64, (g // 2) * 128:(g // 2) * 128 + NBORD] = \
            patR[64 * g:64 * g + 64]
    return (xpad.reshape(2, 128, NPIX).astype(ml_dtypes.bfloat16),
            p1.astype(ml_dtypes.bfloat16),
            np.ascontiguousarray(xi.reshape(2, 128, HW)).astype(np.float32))


def _make_in_maps(x, w1, w2, bn1_gamma, bn1_beta, bn2_gamma, bn2_beta):
    x = np.asarray(x, np.float32)
    ii, jj = _border_indices()
    w1m, w1c = _prep_weights(np.asarray(w1, np.float32))
    w2m, w2c = _prep_weights(np.asarray(w2, np.float32))
    bnp = np.stack([np.asarray(a, np.float32).reshape(2, 128)
                    for a in (bn1_gamma, bn1_beta, bn2_gamma, bn2_beta)])  # [4,2,128]
    bnp = np.ascontiguousarray(bnp.transpose(2, 0, 1).reshape(128, 8))

    in_maps = []
    for c in range(N_CORES):
        xpad, p1, xres = _prep_core(x[c], ii, jj)
        in_maps.append({"xpad": xpad, "xres": xres, "p1": p1,
                        "w1m": w1m, "w1c": w1c, "w2m": w2m, "w2c": w2c,
                        "bnp": bnp})
    return in_maps


def _get_nc():
    if "nc" not in _cached:
        _cached["nc"] = _build_nc()
    return _cached["nc"]


def kernel(x, w1, w2, bn1_gamma, bn1_beta, bn2_gamma, bn2_beta):
    nc = _get_nc()
    in_maps = _make_in_maps(x, w1, w2, bn1_gamma, bn1_beta, bn2_gamma, bn2_beta)
    res = run_bass_kernel_spmd(nc, in_maps, list(range(N_CORES)))
    out = np.stack([res.results[c]["out"].astype(np.float32).reshape(256, 32, 32)
                    for c in range(N_CORES)])
    return out, np.float32(0.0)


def run_traced(inputs):
    """test.py helper: run once with NTFF tracing; returns BassKernelResults."""
    nc = _get_nc()
    in_maps = _make_in_maps(**inputs)
    return run_bass_kernel_spmd(nc, in_maps, list(range(N_CORES)), trace=True)
